# revision 11
# baseline (speedup 1.0000x reference)
"""MHA kernel for TRN2: x[8,512,32,32], 8 heads, S=1024, C=512.

Sharding: data-parallel over batch N=8 -> one batch item per NeuronCore.
Per-core layout (all transpose-free):
  qkT[e,s]  = w_qkvT[:, :1024].T @ x      (e on partitions; q tiles 0-3, k tiles 4-7)
  v[s,e]    = x.T @ w_qkvT[:, 1024:]      (s on partitions, natural layout)
  scoresT   = kT_h.T @ qT_h               (k_s on partitions; K=64 -> head pair packed
                                           at array rows 0-63 / 64-127, runs 2x via
                                           implicit 64x128 row tiling)
  P         = exp(scoresT * 1/8)          (ACT, batched 2048-wide from PSUM)
  oT_aug    = [v_h | 1].T @ P             (M=65; row 64 = softmax denominator r)
  oT        = oT_aug[:64] * (1/r)         (gpsimd partition_broadcast of 1/r)
  yT[o,s]   = w_outT.T @ oT               (+ b_out added host-side; == NCHW layout)

Perf structure vs v1:
  - inputs land in 4 consolidated SBUF tiles (w4/x4/wv4/wo4) loaded by 6 DMAs
    split across the sync + scalar HWDGE queues, ordered so the first matmul's
    operands (w e-slices 0 and 4, x cols 0:512) arrive first
  - 12 warm-up matmuls on a zero tile run during the DMA window so the PE HAM
    clock-gate is at 8/8 before the first real matmul
  - softmax denominators: reciprocal_approx_fast reads PSUM row 64 directly
    (no staging copy), y evacuation runs on DVE - the scalar engine does
    nothing but the 64 exp activations, which are its own hard floor (~63us)
  - final step drains the nt=1 halves first so the second-half output
    projection is not serialized behind all four normalization chains
"""

import numpy as np
import ml_dtypes

import concourse.bacc as bacc
import concourse.mybir as mybir
import concourse.tile as tile
from concourse.bass_utils import run_bass_kernel_spmd

P = 128
S = 1024          # sequence = 32*32
C = 512           # channels
NH = 8            # heads
HD = 64           # head dim
CT = C // P       # 4 c-tiles
ET = 2 * C // P   # 8 e-tiles for q+k
MT = S // P       # 8 s-tiles
BF = mybir.dt.bfloat16
F32 = mybir.dt.float32

_cache = {}


def build_program(dbg=False):
    nc = bacc.Bacc("TRN2", target_bir_lowering=False, debug=False, num_devices=8)
    # inputs come pre-swizzled from the host so every DMA is a contiguous
    # >=2KB-per-partition read (small descriptors collapse queue throughput):
    #   x:  [p, nt, ct, 512]  = x[ct*128+p, nt*512+s]
    #   wq: [p, eo, ct, 128]  = w_qkvT[ct*128+p, ET_ORDER[eo]*128+e]
    #   wv: [p, ct, 512]      = w_qkvT[ct*128+p, 1024+e]
    #   wo: [p, ct, 512]      = w_outT[ct*128+p, o]
    x_d = nc.dram_tensor("x", [P, 2 * CT * 512], BF, kind="ExternalInput").ap()
    wq_d = nc.dram_tensor("wq", [P, ET * CT * P], BF, kind="ExternalInput").ap()
    wv_d = nc.dram_tensor("wv", [P, CT * C], BF, kind="ExternalInput").ap()
    wo_d = nc.dram_tensor("wo", [P, CT * C], BF, kind="ExternalInput").ap()
    y_d = nc.dram_tensor("y", [C, S], BF, kind="ExternalOutput").ap()

    with tile.TileContext(nc) as tc:
        with (
            tc.tile_pool(name="const", bufs=1) as cpool,
            tc.tile_pool(name="qk", bufs=1) as qkpool,
            tc.tile_pool(name="vp", bufs=1) as vpool,
            tc.tile_pool(name="pp", bufs=32) as ppool,
            tc.tile_pool(name="ot", bufs=1) as opool,
            tc.tile_pool(name="yp", bufs=1) as ypool,
            tc.tile_pool(name="misc", bufs=4) as mpool,
            tc.tile_pool(name="psq", bufs=2, space="PSUM") as psq_pool,
            tc.tile_pool(name="pso", bufs=4, space="PSUM") as pso_pool,
        ):
            # ---- warm-up: zero tile + dummy matmuls keep the PE busy while
            # input DMAs stream, so HAM is un-throttled for the real work ----
            wz = cpool.tile([P, 512], BF, name="wz", tag="wz")
            nc.gpsimd.memset(wz[:], 0.0)
            wu = psq_pool.tile([P, 1024], F32, name="wu", tag="psq")
            for _ in range(6):
                nc.tensor.matmul(wu[:, 0:512], wz[:, 0:128], wz[:], start=True, stop=True)

            # ---- consolidated input tiles, same layout as the prepped DRAM ----
            w4 = cpool.tile([P, ET * CT * P], BF, name="w4", tag="w4")
            x4 = cpool.tile([P, 2 * CT * 512], BF, name="x4", tag="x4")
            wv4 = cpool.tile([P, CT * C], BF, name="wv4", tag="wv4")
            wo4 = cpool.tile([P, CT * C], BF, name="wo4", tag="wo4")

            # sync queue: everything the first matmul groups need, in order;
            # scalar queue: the rest. All reads are contiguous per partition.
            nc.sync.dma_start(w4[:, 0:1024], wq_d[:, 0:1024])      # et 0 and 4
            nc.sync.dma_start(x4[:, 0:2048], x_d[:, 0:2048])       # s 0:512
            nc.scalar.dma_start(w4[:, 1024:4096], wq_d[:, 1024:4096])
            nc.scalar.dma_start(x4[:, 2048:4096], x_d[:, 2048:4096])  # s 512:1024
            nc.scalar.dma_start(wv4[:], wv_d[:])
            nc.scalar.dma_start(wo4[:], wo_d[:])

            ET_ORDER = (0, 4, 1, 5, 2, 6, 3, 7)
            ET_OFF = {et: i for i, et in enumerate(ET_ORDER)}

            def w_slice(et, ct):
                o = (ET_OFF[et] * CT + ct) * P
                return w4[:, o:o + P]

            def x_nt(ct, nt):
                o = (nt * CT + ct) * 512
                return x4[:, o:o + 512]

            def x_mt(ct, mt):
                nt, r = divmod(mt, 4)
                o = (nt * CT + ct) * 512 + r * P
                return x4[:, o:o + P]

            def wv_sb(ct):
                return wv4[:, ct * C:(ct + 1) * C]

            def wo_sb(ct):
                return wo4[:, ct * C:(ct + 1) * C]

            # ---- qkT projection: [e=1024 rows, s=1024] ----
            qk_sb = []
            for et in range(ET):
                t = qkpool.tile([P, S], BF, name=f"qk{et}", tag=f"qk{et}")
                qk_sb.append(t)
            v_sb = [None] * MT

            def emit_qkv_group(et, nt):
                ps = pso_pool.tile([P, 512], F32, name="qp", tag="pso")
                for ct in range(CT):
                    nc.tensor.matmul(
                        ps[:],
                        w_slice(et, ct),
                        x_nt(ct, nt),
                        start=(ct == 0), stop=(ct == CT - 1),
                    )
                nc.vector.tensor_copy(qk_sb[et][:, nt * 512:(nt + 1) * 512], ps[:])

            def emit_v_group(mt):
                ps = pso_pool.tile([P, 512], F32, name="vp", tag="pso")
                for ct in range(CT):
                    nc.tensor.matmul(
                        ps[:],
                        x_mt(ct, mt),
                        wv_sb(ct),
                        start=(ct == 0), stop=(ct == CT - 1),
                    )
                vt = vpool.tile([P, NH * (HD + 1)], BF, name=f"v{mt}", tag=f"v{mt}")
                vv = vt[:].rearrange("p (h e) -> p h e", e=HD + 1)
                nc.gpsimd.memset(vv[:, :, HD:HD + 1], 1.0)
                nc.vector.tensor_copy(
                    vv[:, :, 0:HD], ps[:].rearrange("p (h e) -> p h e", e=HD))
                v_sb[mt] = vt

            # block A: only the nt0 tiles gate the first QK/exp; the nt1
            # groups are emitted right after the first ACT (see step loop)
            for et, nt in ((0, 0), (4, 0)):
                emit_qkv_group(et, nt)
            pending = [("qkv", et, nt) for et in (1, 5, 2, 6, 3, 7) for nt in (0, 1)]
            pending += [("v", mt, None) for mt in range(MT)]
            pend_i = 0

            # ---- attention, software-pipelined: QK/exp(pair p) || PV(pair p-1);
            #      step 0 also drains the remaining qkv/v projection groups ----
            oT_sb = [opool.tile([P, S], BF, name=f"o{ct}", tag=f"o{ct}") for ct in range(CT)]
            p_tiles = {}
            DRAIN_ORDER = ((0, 0), (1, 0), (0, 1), (1, 1))
            LAST_ORDER = ((0, 1), (1, 1), (0, 0), (1, 0))

            def emit_norm(step, idx_order_idx, pso_t, order):
                # normalize oT slice for drain index `idx` of pair step-1
                pp = step - 1
                hh, nt = order[idx_order_idx]
                h = 2 * pp + hh
                ct, half = h // 2, h % 2
                rrow = mpool.tile([1, 512], F32, name="rrow", tag="rrow")
                nc.vector.tensor_copy(rrow[0:1, :], pso_t[idx_order_idx][HD:HD + 1, :])
                rinv = mpool.tile([1, 512], F32, name="rinv", tag="rinv")
                nc.vector.reciprocal_approx_fast(rinv[0:1, :], rrow[0:1, :])
                bc = mpool.tile([HD, 512], F32, name="bc", tag="bc")
                nc.gpsimd.partition_broadcast(bc[:], rinv[0:1, :], channels=HD)
                nc.vector.tensor_mul(
                    oT_sb[ct][half * HD:(half + 1) * HD, nt * 512:(nt + 1) * 512],
                    pso_t[idx_order_idx][0:HD, :], bc[:],
                )

            y_sb = [ypool.tile([P, S], BF, name=f"y{ot}", tag=f"y{ot}") for ot in range(CT)]

            for step in range(NH // 2 + 1):
                last = step == NH // 2
                order = LAST_ORDER if last else DRAIN_ORDER
                pso_t = None
                if step >= 1:
                    pso_t = [pso_pool.tile([P, 512], F32, name=f"pso{i}", tag="pso")
                             for i in range(4)]
                for mt in range(MT):
                    if step < NH // 2:
                        for nt in range(2):
                            psq = psq_pool.tile([P, 1024], F32, name="psq", tag="psq")
                            for hh in range(2):
                                nc.tensor.matmul(
                                    psq[:, hh * 512:(hh + 1) * 512],
                                    qk_sb[4 + step][hh * HD:(hh + 1) * HD, mt * P:(mt + 1) * P],
                                    qk_sb[step][hh * HD:(hh + 1) * HD, nt * 512:(nt + 1) * 512],
                                    start=True, stop=True,
                                )
                            pt = ppool.tile([P, 1024], BF, name="ptile", tag="ptile")
                            nc.scalar.activation(
                                pt[:], psq[:], mybir.ActivationFunctionType.Exp,
                                scale=float(1.0 / np.sqrt(HD)),
                            )
                            p_tiles[(step, mt, nt)] = pt
                            if step == 0 and mt == 0 and nt == 0:
                                emit_qkv_group(0, 1)
                                emit_qkv_group(4, 1)
                            if step == 0:
                                slot = mt * 2 + nt
                                want = 20 * (slot + 1) // 16
                                while pend_i < min(want, 20):
                                    kind, i1, i2 = pending[pend_i]
                                    if kind == "qkv":
                                        emit_qkv_group(i1, i2)
                                    else:
                                        emit_v_group(i1)
                                    pend_i += 1
                    if step >= 1:
                        pp = step - 1
                        for idx, (hh, nt) in enumerate(order):
                            h = 2 * pp + hh
                            nc.tensor.matmul(
                                pso_t[idx][0:HD + 1, :],
                                v_sb[mt][:, h * (HD + 1):(h + 1) * (HD + 1)],
                                p_tiles[(pp, mt, nt)][:, hh * 512:(hh + 1) * 512],
                                start=(mt == 0), stop=(mt == MT - 1),
                            )
                if step >= 1 and not last:
                    for i in range(4):
                        emit_norm(step, i, pso_t, order)

            # ---- final step normalization + output projection ----
            # LAST_ORDER puts the nt=1 halves first: the st1 projection (q
            # columns 512:1024) only waits on those two chains, while st0
            # (columns 0:512, on the freed psq ring) runs underneath.
            psA = psq_pool.tile([P, 1024], F32, name="prA", tag="psq")
            psB = psq_pool.tile([P, 1024], F32, name="prB", tag="psq")

            def st0_ps(g):
                t = psA if g < 2 else psB
                return t[:, (g % 2) * 512:(g % 2 + 1) * 512]

            # st0 accumulation over ct=0..2 has no dependency on the last pair
            for ct in range(CT - 1):
                for g in range(CT):
                    nc.tensor.matmul(
                        st0_ps(g),
                        wo_sb(ct)[:, g * P:(g + 1) * P],
                        oT_sb[ct][:, 0:512],
                        start=(ct == 0), stop=False,
                    )

            # nt=1 normalization chains, then st1 projection. st1 groups g
            # reuse the pso ring slots of pso_t[g] in order, so g0/g1 come
            # after the first two chains and g2/g3 after the last two
            # (emission order must match or the in-order DVE deadlocks).
            def emit_st1(g):
                ps = pso_pool.tile([P, 512], F32, name="op", tag="pso")
                for ct in range(CT):
                    nc.tensor.matmul(
                        ps[:],
                        wo_sb(ct)[:, g * P:(g + 1) * P],
                        oT_sb[ct][:, 512:1024],
                        start=(ct == 0), stop=(ct == CT - 1),
                    )
                dst = y_sb[g][:, 512:1024]
                nc.scalar.copy(dst, ps[:])
                (nc.sync if g % 2 == 0 else nc.scalar).dma_start(
                    y_d[g * P:(g + 1) * P, 512:1024], dst)

            emit_norm(NH // 2, 0, pso_t, LAST_ORDER)
            emit_norm(NH // 2, 1, pso_t, LAST_ORDER)
            emit_st1(0)
            emit_st1(1)

            # nt=0 normalization chains, close st0, evacuate
            emit_norm(NH // 2, 2, pso_t, LAST_ORDER)
            emit_norm(NH // 2, 3, pso_t, LAST_ORDER)
            emit_st1(2)
            emit_st1(3)
            ct = CT - 1
            for g in range(CT):
                nc.tensor.matmul(
                    st0_ps(g),
                    wo_sb(ct)[:, g * P:(g + 1) * P],
                    oT_sb[ct][:, 0:512],
                    start=False, stop=True,
                )
            for g in range(CT):
                dst = y_sb[g][:, 0:512]
                nc.scalar.copy(dst, st0_ps(g))
                (nc.sync if g % 2 == 0 else nc.scalar).dma_start(
                    y_d[g * P:(g + 1) * P, 0:512], dst)

    nc.compile()
    return nc


def get_program():
    if "nc" not in _cache:
        _cache["nc"] = build_program()
    return _cache["nc"]


def kernel(x, w_qkv, w_out, b_out, _trace=False, _tmpdir=None):
    x = np.asarray(x, dtype=np.float32)
    w_qkv = np.asarray(w_qkv, dtype=np.float32)
    w_out = np.asarray(w_out, dtype=np.float32)
    b_out = np.asarray(b_out, dtype=np.float32)
    N = x.shape[0]

    xb = x.reshape(N, C, S).astype(ml_dtypes.bfloat16)
    wqT = np.ascontiguousarray(w_qkv.T).astype(ml_dtypes.bfloat16)
    woT = np.ascontiguousarray(w_out.T).astype(ml_dtypes.bfloat16)

    # pre-swizzle into the contiguous-per-partition layouts the kernel DMAs
    ET_ORDER = [0, 4, 1, 5, 2, 6, 3, 7]
    # wq: [ct,p,et,e'] -> [p, eo, ct, e']
    wq_pre = np.ascontiguousarray(
        wqT[:, :1024].reshape(CT, P, ET, P).transpose(1, 2, 0, 3)[:, ET_ORDER]
    ).reshape(P, ET * CT * P)
    wv_pre = np.ascontiguousarray(
        wqT[:, 1024:1536].reshape(CT, P, C).transpose(1, 0, 2)
    ).reshape(P, CT * C)
    wo_pre = np.ascontiguousarray(
        woT.reshape(CT, P, C).transpose(1, 0, 2)
    ).reshape(P, CT * C)
    # x: [ct,p,nt,s'] -> [p, nt, ct, s']
    x_pre = [
        np.ascontiguousarray(
            xb[n].reshape(CT, P, 2, 512).transpose(1, 2, 0, 3)
        ).reshape(P, 2 * CT * 512)
        for n in range(N)
    ]

    nc = get_program()
    in_maps = [
        {"x": x_pre[n], "wq": wq_pre, "wv": wv_pre, "wo": wo_pre}
        for n in range(N)
    ]
    res = run_bass_kernel_spmd(
        nc, in_maps, core_ids=list(range(N)), trace=_trace, tmpdir=_tmpdir
    )
    y = np.stack([res.results[n]["y"] for n in range(N)]).astype(np.float32)
    y = y.reshape(N, C, 32, 32)
    y = y + b_out[None, :, None, None]
    if _trace:
        return y, res
    return y


# revision 14
# speedup vs baseline: 1.0216x; 1.0216x over previous
"""MHA kernel for TRN2: x[8,512,32,32], 8 heads, S=1024, C=512.

Sharding: data-parallel over batch N=8 -> one batch item per NeuronCore.
Per-core layout (all transpose-free):
  qkT[e,s]  = w_qkvT[:, :1024].T @ x      (e on partitions; q tiles 0-3, k tiles 4-7)
  v[s,e]    = x.T @ w_qkvT[:, 1024:]      (s on partitions, natural layout)
  scoresT   = kT_h.T @ qT_h               (k_s on partitions; K=64 -> head pair packed
                                           at array rows 0-63 / 64-127, runs 2x via
                                           implicit 64x128 row tiling)
  P         = exp(scoresT * 1/8)          (ACT, batched 2048-wide from PSUM)
  oT_aug    = [v_h | 1].T @ P             (M=65; row 64 = softmax denominator r)
  oT        = oT_aug[:64] * (1/r)         (gpsimd partition_broadcast of 1/r)
  yT[o,s]   = w_outT.T @ oT               (+ b_out added host-side; == NCHW layout)

Perf structure vs v1:
  - inputs land in 4 consolidated SBUF tiles (w4/x4/wv4/wo4) loaded by 6 DMAs
    split across the sync + scalar HWDGE queues, ordered so the first matmul's
    operands (w e-slices 0 and 4, x cols 0:512) arrive first
  - 12 warm-up matmuls on a zero tile run during the DMA window so the PE HAM
    clock-gate is at 8/8 before the first real matmul
  - softmax denominators: reciprocal_approx_fast reads PSUM row 64 directly
    (no staging copy), y evacuation runs on DVE - the scalar engine does
    nothing but the 64 exp activations, which are its own hard floor (~63us)
  - final step drains the nt=1 halves first so the second-half output
    projection is not serialized behind all four normalization chains
"""

import numpy as np
import ml_dtypes

import concourse.bacc as bacc
import concourse.mybir as mybir
import concourse.tile as tile
from concourse.bass_utils import run_bass_kernel_spmd

P = 128
S = 1024          # sequence = 32*32
C = 512           # channels
NH = 8            # heads
HD = 64           # head dim
CT = C // P       # 4 c-tiles
ET = 2 * C // P   # 8 e-tiles for q+k
MT = S // P       # 8 s-tiles
BF = mybir.dt.bfloat16
F32 = mybir.dt.float32

_cache = {}


def build_program(dbg=False):
    nc = bacc.Bacc("TRN2", target_bir_lowering=False, debug=False, num_devices=8)
    # inputs come pre-swizzled from the host so every DMA is a contiguous
    # >=2KB-per-partition read (small descriptors collapse queue throughput):
    #   x:  [p, nt, ct, 512]  = x[ct*128+p, nt*512+s]
    #   wq: [p, eo, ct, 128]  = w_qkvT[ct*128+p, ET_ORDER[eo]*128+e]
    #   wv: [p, ct, 512]      = w_qkvT[ct*128+p, 1024+e]
    #   wo: [p, ct, 512]      = w_outT[ct*128+p, o]
    xa_d = nc.dram_tensor("xa", [P, CT * 512], BF, kind="ExternalInput").ap()
    xb_d = nc.dram_tensor("xb", [P, CT * 512], BF, kind="ExternalInput").ap()
    wqa_d = nc.dram_tensor("wqa", [P, 2 * CT * P], BF, kind="ExternalInput").ap()
    wqb_d = nc.dram_tensor("wqb", [P, 6 * CT * P], BF, kind="ExternalInput").ap()
    wv_d = nc.dram_tensor("wv", [P, CT * C], BF, kind="ExternalInput").ap()
    wo_d = nc.dram_tensor("wo", [P, CT * C], BF, kind="ExternalInput").ap()
    y_d = nc.dram_tensor("y", [C, S], BF, kind="ExternalOutput").ap()

    with tile.TileContext(nc) as tc:
        with (
            tc.tile_pool(name="const", bufs=1) as cpool,
            tc.tile_pool(name="qk", bufs=1) as qkpool,
            tc.tile_pool(name="vp", bufs=1) as vpool,
            tc.tile_pool(name="pp", bufs=32) as ppool,
            tc.tile_pool(name="ot", bufs=1) as opool,
            tc.tile_pool(name="yp", bufs=1) as ypool,
            tc.tile_pool(name="misc", bufs=4) as mpool,
            tc.tile_pool(name="psq", bufs=2, space="PSUM") as psq_pool,
            tc.tile_pool(name="pso", bufs=4, space="PSUM") as pso_pool,
        ):
            # ---- warm-up: zero tile + dummy matmuls keep the PE busy while
            # input DMAs stream, so HAM is un-throttled for the real work ----
            wz = cpool.tile([P, 512], BF, name="wz", tag="wz")
            nc.gpsimd.memset(wz[:], 0.0)
            wu = psq_pool.tile([P, 1024], F32, name="wu", tag="psq")
            for _ in range(6):
                nc.tensor.matmul(wu[:, 0:512], wz[:, 0:128], wz[:], start=True, stop=True)

            # ---- consolidated input tiles, same layout as the prepped DRAM ----
            w4 = cpool.tile([P, ET * CT * P], BF, name="w4", tag="w4")
            x4 = cpool.tile([P, 2 * CT * 512], BF, name="x4", tag="x4")
            wv4 = cpool.tile([P, CT * C], BF, name="wv4", tag="wv4")
            wo4 = cpool.tile([P, CT * C], BF, name="wo4", tag="wo4")

            # sync queue: everything the first matmul groups need, in order;
            # scalar queue: the rest. Each DMA reads one fully-contiguous
            # DRAM tensor (strided sources run the queue ~2.5x slower).
            nc.sync.dma_start(w4[:, 0:1024], wqa_d[:])             # et 0 and 4
            nc.sync.dma_start(x4[:, 0:2048], xa_d[:])              # s 0:512
            nc.sync.dma_start(w4[:, 1024:4096], wqb_d[:])
            nc.scalar.dma_start(x4[:, 2048:4096], xb_d[:])         # s 512:1024
            nc.scalar.dma_start(wv4[:], wv_d[:])
            nc.scalar.dma_start(wo4[:], wo_d[:])

            ET_ORDER = (0, 4, 1, 5, 2, 6, 3, 7)
            ET_OFF = {et: i for i, et in enumerate(ET_ORDER)}

            def w_slice(et, ct):
                o = (ET_OFF[et] * CT + ct) * P
                return w4[:, o:o + P]

            def x_nt(ct, nt):
                o = (nt * CT + ct) * 512
                return x4[:, o:o + 512]

            def x_mt(ct, mt):
                nt, r = divmod(mt, 4)
                o = (nt * CT + ct) * 512 + r * P
                return x4[:, o:o + P]

            def wv_sb(ct):
                return wv4[:, ct * C:(ct + 1) * C]

            def wo_sb(ct):
                return wo4[:, ct * C:(ct + 1) * C]

            # ---- qkT projection: [e=1024 rows, s=1024] ----
            qk_sb = []
            for et in range(ET):
                t = qkpool.tile([P, S], BF, name=f"qk{et}", tag=f"qk{et}")
                qk_sb.append(t)
            v_sb = [None] * MT

            def emit_qkv_group(et, nt):
                ps = pso_pool.tile([P, 512], F32, name="qp", tag="pso")
                for ct in range(CT):
                    nc.tensor.matmul(
                        ps[:],
                        w_slice(et, ct),
                        x_nt(ct, nt),
                        start=(ct == 0), stop=(ct == CT - 1),
                    )
                nc.vector.tensor_copy(qk_sb[et][:, nt * 512:(nt + 1) * 512], ps[:])

            def emit_v_group(mt):
                ps = pso_pool.tile([P, 512], F32, name="vp", tag="pso")
                for ct in range(CT):
                    nc.tensor.matmul(
                        ps[:],
                        x_mt(ct, mt),
                        wv_sb(ct),
                        start=(ct == 0), stop=(ct == CT - 1),
                    )
                vt = vpool.tile([P, NH * (HD + 1)], BF, name=f"v{mt}", tag=f"v{mt}")
                vv = vt[:].rearrange("p (h e) -> p h e", e=HD + 1)
                nc.gpsimd.memset(vv[:, :, HD:HD + 1], 1.0)
                nc.vector.tensor_copy(
                    vv[:, :, 0:HD], ps[:].rearrange("p (h e) -> p h e", e=HD))
                v_sb[mt] = vt

            # block A: only the nt0 tiles gate the first QK/exp; the nt1
            # groups are emitted right after the first ACT (see step loop)
            for et, nt in ((0, 0), (4, 0)):
                emit_qkv_group(et, nt)
            pending = [("qkv", et, nt) for et in (1, 5, 2, 6, 3, 7) for nt in (0, 1)]
            pending += [("v", mt, None) for mt in range(MT)]
            pend_i = 0

            # ---- attention, software-pipelined: QK/exp(pair p) || PV(pair p-1);
            #      step 0 also drains the remaining qkv/v projection groups ----
            oT_sb = [opool.tile([P, S], BF, name=f"o{ct}", tag=f"o{ct}") for ct in range(CT)]
            p_tiles = {}
            DRAIN_ORDER = ((0, 0), (1, 0), (0, 1), (1, 1))
            LAST_ORDER = ((0, 1), (1, 1), (0, 0), (1, 0))

            def emit_norm(step, idx_order_idx, pso_t, order):
                # normalize oT slice for drain index `idx` of pair step-1
                pp = step - 1
                hh, nt = order[idx_order_idx]
                h = 2 * pp + hh
                ct, half = h // 2, h % 2
                rrow = mpool.tile([1, 512], F32, name="rrow", tag="rrow")
                nc.vector.tensor_copy(rrow[0:1, :], pso_t[idx_order_idx][HD:HD + 1, :])
                rinv = mpool.tile([1, 512], F32, name="rinv", tag="rinv")
                nc.vector.reciprocal_approx_fast(rinv[0:1, :], rrow[0:1, :])
                bc = mpool.tile([HD, 512], F32, name="bc", tag="bc")
                nc.gpsimd.partition_broadcast(bc[:], rinv[0:1, :], channels=HD)
                nc.vector.tensor_mul(
                    oT_sb[ct][half * HD:(half + 1) * HD, nt * 512:(nt + 1) * 512],
                    pso_t[idx_order_idx][0:HD, :], bc[:],
                )

            y_sb = [ypool.tile([P, S], BF, name=f"y{ot}", tag=f"y{ot}") for ot in range(CT)]

            for step in range(NH // 2 + 1):
                last = step == NH // 2
                order = LAST_ORDER if last else DRAIN_ORDER
                pso_t = None
                if step >= 1:
                    pso_t = [pso_pool.tile([P, 512], F32, name=f"pso{i}", tag="pso")
                             for i in range(4)]
                for mt in range(MT):
                    if step < NH // 2:
                        for nt in range(2):
                            psq = psq_pool.tile([P, 1024], F32, name="psq", tag="psq")
                            for hh in range(2):
                                nc.tensor.matmul(
                                    psq[:, hh * 512:(hh + 1) * 512],
                                    qk_sb[4 + step][hh * HD:(hh + 1) * HD, mt * P:(mt + 1) * P],
                                    qk_sb[step][hh * HD:(hh + 1) * HD, nt * 512:(nt + 1) * 512],
                                    start=True, stop=True,
                                )
                            pt = ppool.tile([P, 1024], BF, name="ptile", tag="ptile")
                            nc.scalar.activation(
                                pt[:], psq[:], mybir.ActivationFunctionType.Exp,
                                scale=float(1.0 / np.sqrt(HD)),
                            )
                            p_tiles[(step, mt, nt)] = pt
                            if step == 0 and mt == 0 and nt == 0:
                                emit_qkv_group(0, 1)
                                emit_qkv_group(4, 1)
                            if step == 0:
                                slot = mt * 2 + nt
                                want = 20 * (slot + 1) // 16
                                while pend_i < min(want, 20):
                                    kind, i1, i2 = pending[pend_i]
                                    if kind == "qkv":
                                        emit_qkv_group(i1, i2)
                                    else:
                                        emit_v_group(i1)
                                    pend_i += 1
                    if step >= 1:
                        pp = step - 1
                        for idx, (hh, nt) in enumerate(order):
                            h = 2 * pp + hh
                            nc.tensor.matmul(
                                pso_t[idx][0:HD + 1, :],
                                v_sb[mt][:, h * (HD + 1):(h + 1) * (HD + 1)],
                                p_tiles[(pp, mt, nt)][:, hh * 512:(hh + 1) * 512],
                                start=(mt == 0), stop=(mt == MT - 1),
                            )
                if step >= 1 and not last:
                    for i in range(4):
                        emit_norm(step, i, pso_t, order)

            # ---- final step normalization + output projection ----
            # LAST_ORDER puts the nt=1 halves first: the st1 projection (q
            # columns 512:1024) only waits on those two chains, while st0
            # (columns 0:512, on the freed psq ring) runs underneath.
            psA = psq_pool.tile([P, 1024], F32, name="prA", tag="psq")
            psB = psq_pool.tile([P, 1024], F32, name="prB", tag="psq")

            def st0_ps(g):
                t = psA if g < 2 else psB
                return t[:, (g % 2) * 512:(g % 2 + 1) * 512]

            # st0 accumulation over ct=0..2 has no dependency on the last pair
            for ct in range(CT - 1):
                for g in range(CT):
                    nc.tensor.matmul(
                        st0_ps(g),
                        wo_sb(ct)[:, g * P:(g + 1) * P],
                        oT_sb[ct][:, 0:512],
                        start=(ct == 0), stop=False,
                    )

            # nt=1 normalization chains, then st1 projection. st1 groups g
            # reuse the pso ring slots of pso_t[g] in order, so g0/g1 come
            # after the first two chains and g2/g3 after the last two
            # (emission order must match or the in-order DVE deadlocks).
            def emit_st1(g):
                ps = pso_pool.tile([P, 512], F32, name="op", tag="pso")
                for ct in range(CT):
                    nc.tensor.matmul(
                        ps[:],
                        wo_sb(ct)[:, g * P:(g + 1) * P],
                        oT_sb[ct][:, 512:1024],
                        start=(ct == 0), stop=(ct == CT - 1),
                    )
                dst = y_sb[g][:, 512:1024]
                nc.scalar.copy(dst, ps[:])
                (nc.sync if g % 2 == 0 else nc.scalar).dma_start(
                    y_d[g * P:(g + 1) * P, 512:1024], dst)

            emit_norm(NH // 2, 0, pso_t, LAST_ORDER)
            emit_norm(NH // 2, 1, pso_t, LAST_ORDER)
            emit_st1(0)
            emit_st1(1)

            # nt=0 normalization chains, close st0, evacuate
            emit_norm(NH // 2, 2, pso_t, LAST_ORDER)
            emit_norm(NH // 2, 3, pso_t, LAST_ORDER)
            emit_st1(2)
            emit_st1(3)
            ct = CT - 1
            for g in range(CT):
                nc.tensor.matmul(
                    st0_ps(g),
                    wo_sb(ct)[:, g * P:(g + 1) * P],
                    oT_sb[ct][:, 0:512],
                    start=False, stop=True,
                )
            for g in range(CT):
                dst = y_sb[g][:, 0:512]
                nc.scalar.copy(dst, st0_ps(g))
                (nc.sync if g % 2 == 0 else nc.scalar).dma_start(
                    y_d[g * P:(g + 1) * P, 0:512], dst)

    nc.compile()
    return nc


def get_program():
    if "nc" not in _cache:
        _cache["nc"] = build_program()
    return _cache["nc"]


def kernel(x, w_qkv, w_out, b_out, _trace=False, _tmpdir=None):
    x = np.asarray(x, dtype=np.float32)
    w_qkv = np.asarray(w_qkv, dtype=np.float32)
    w_out = np.asarray(w_out, dtype=np.float32)
    b_out = np.asarray(b_out, dtype=np.float32)
    N = x.shape[0]

    xb = x.reshape(N, C, S).astype(ml_dtypes.bfloat16)
    wqT = np.ascontiguousarray(w_qkv.T).astype(ml_dtypes.bfloat16)
    woT = np.ascontiguousarray(w_out.T).astype(ml_dtypes.bfloat16)

    # pre-swizzle into the contiguous-per-partition layouts the kernel DMAs
    ET_ORDER = [0, 4, 1, 5, 2, 6, 3, 7]
    # wq: [ct,p,et,e'] -> [p, eo, ct, e']
    wq_pre = np.ascontiguousarray(
        wqT[:, :1024].reshape(CT, P, ET, P).transpose(1, 2, 0, 3)[:, ET_ORDER]
    ).reshape(P, ET * CT * P)
    wqa_pre = np.ascontiguousarray(wq_pre[:, :1024])
    wqb_pre = np.ascontiguousarray(wq_pre[:, 1024:])
    wv_pre = np.ascontiguousarray(
        wqT[:, 1024:1536].reshape(CT, P, C).transpose(1, 0, 2)
    ).reshape(P, CT * C)
    wo_pre = np.ascontiguousarray(
        woT.reshape(CT, P, C).transpose(1, 0, 2)
    ).reshape(P, CT * C)
    # x: [ct,p,nt,s'] -> [p, nt, ct, s']
    x_pre = [
        np.ascontiguousarray(
            xb[n].reshape(CT, P, 2, 512).transpose(1, 2, 0, 3)
        ).reshape(P, 2 * CT * 512)
        for n in range(N)
    ]

    nc = get_program()
    in_maps = [
        {
            "xa": np.ascontiguousarray(x_pre[n][:, :2048]),
            "xb": np.ascontiguousarray(x_pre[n][:, 2048:]),
            "wqa": wqa_pre, "wqb": wqb_pre, "wv": wv_pre, "wo": wo_pre,
        }
        for n in range(N)
    ]
    res = run_bass_kernel_spmd(
        nc, in_maps, core_ids=list(range(N)), trace=_trace, tmpdir=_tmpdir
    )
    y = np.stack([res.results[n]["y"] for n in range(N)]).astype(np.float32)
    y = y.reshape(N, C, 32, 32)
    y = y + b_out[None, :, None, None]
    if _trace:
        return y, res
    return y
